# revision 2
# baseline (speedup 1.0000x reference)
"""Trainium2 Bass kernel for CrossDMHAttention (B=131072, single-query cross-attention
with T=24 kv tokens, H=4 heads, head_dim=8, + LN + residual GELU MLP).

Strategy: pure data-parallel over 8 NeuronCores (batch split). Per core, 128-row
tiles in natural [batch-partition, feature-free] layout:
  - PE transposes each 128x128 chunk of kv; block-diagonal weights project k,v for
    4 tokens per matmul (PSUM).
  - DVE computes per-row score/context contractions with broadcast APs + strided
    reduces; ACT does exp/gelu/copies; PE does all projections.
Weights are tiny and replicated; scale 1/sqrt(8) folded into Wq.
"""

import math

import numpy as np

B, DQ, DKV, T, A, H, O = 131072, 13, 32, 24, 32, 4, 32
HD = A // H
LN_EPS = 1e-5
NCORES = 8
BP = B // NCORES  # rows per core
P = 128
NT = BP // P      # tiles per core

_CACHE = {}


def _ap(base, dims, extra_offset=0):
    """Build an AP over base (an AP) with given free [step, count] dims."""
    import concourse.bass as bass
    return bass.AP(tensor=base.tensor, offset=base.offset + extra_offset,
                   ap=[base.ap[0]] + [list(d) for d in dims])


def _build():
    import concourse.bacc as bacc
    import concourse.tile as tile
    from concourse import mybir

    f32 = mybir.dt.float32
    AF = mybir.ActivationFunctionType
    OP = mybir.AluOpType
    AX = mybir.AxisListType

    nc = bacc.Bacc()

    q_d = nc.dram_tensor("q_in", [BP, DQ], f32, kind="ExternalInput")
    kv_d = nc.dram_tensor("kv_in", [BP, T * DKV], f32, kind="ExternalInput")
    ident_d = nc.dram_tensor("ident", [128, 128], f32, kind="ExternalInput")
    wq_d = nc.dram_tensor("wq", [16, 32], f32, kind="ExternalInput")
    wkv_d = nc.dram_tensor("wkv", [128, 256], f32, kind="ExternalInput")
    wo_d = nc.dram_tensor("wo", [32, 32], f32, kind="ExternalInput")
    wd1_d = nc.dram_tensor("wd1", [32, 32], f32, kind="ExternalInput")
    wd2_d = nc.dram_tensor("wd2", [32, 32], f32, kind="ExternalInput")
    lnw_d = nc.dram_tensor("lnw", [128, 32], f32, kind="ExternalInput")
    lnb_d = nc.dram_tensor("lnb", [128, 32], f32, kind="ExternalInput")
    out_d = nc.dram_tensor("out", [BP, O], f32, kind="ExternalOutput")

    with tile.TileContext(nc) as tc:
        with (
            tc.tile_pool(name="consts", bufs=1) as consts,
            tc.tile_pool(name="kvload", bufs=3) as kvload,
            tc.tile_pool(name="bigs", bufs=2) as bigs,
            tc.tile_pool(name="trsb", bufs=3) as trsb,
            tc.tile_pool(name="smalls", bufs=2) as smalls,
            tc.tile_pool(name="kvpp", bufs=1, space="PSUM") as kvpp,
            tc.tile_pool(name="qpp", bufs=1, space="PSUM") as qpp,
            tc.tile_pool(name="tps", bufs=3, space="PSUM") as tps,
            tc.tile_pool(name="mms", bufs=1, space="PSUM") as mms,
        ):
            ident_sb = consts.tile([128, 128], f32)
            wq_sb = consts.tile([16, 32], f32)
            wkv_sb = consts.tile([128, 256], f32)
            wo_sb = consts.tile([32, 32], f32)
            wd1_sb = consts.tile([32, 32], f32)
            wd2_sb = consts.tile([32, 32], f32)
            lnw_sb = consts.tile([128, 32], f32)
            lnb_sb = consts.tile([128, 32], f32)
            eps_sb = consts.tile([128, 1], f32)
            q_all = consts.tile([128, NT, 16], f32)

            nc.sync.dma_start(out=ident_sb, in_=ident_d[:, :])
            nc.sync.dma_start(out=wq_sb, in_=wq_d[:, :])
            nc.sync.dma_start(out=wkv_sb, in_=wkv_d[:, :])
            nc.sync.dma_start(out=wo_sb, in_=wo_d[:, :])
            nc.sync.dma_start(out=wd1_sb, in_=wd1_d[:, :])
            nc.sync.dma_start(out=wd2_sb, in_=wd2_d[:, :])
            nc.sync.dma_start(out=lnw_sb, in_=lnw_d[:, :])
            nc.sync.dma_start(out=lnb_sb, in_=lnb_d[:, :])
            nc.vector.memset(eps_sb, LN_EPS)
            nc.gpsimd.memset(q_all, 0.0)
            nc.sync.dma_start(
                out=q_all[:, :, 0:DQ],
                in_=q_d.rearrange("(i p) c -> p i c", p=P),
            )

            for i in range(NT):
                kv_t = kvload.tile([P, T * DKV], f32, tag="kv")
                nc.sync.dma_start(out=kv_t, in_=kv_d[i * P:(i + 1) * P, :])

                # ---- q projection (scale folded into wq) ----
                qT_ps = tps.tile([16, 128], f32, tag="tp")
                nc.tensor.transpose(qT_ps, q_all[:, i, :], ident_sb)
                qT_sb = trsb.tile([16, 128], f32, tag="qT")
                nc.scalar.copy(qT_sb, qT_ps)
                q_ps = qpp.tile([P, 32], f32, tag="qp")
                nc.tensor.matmul(q_ps, lhsT=qT_sb, rhs=wq_sb)
                q_sb = smalls.tile([P, 32], f32, tag="qsb")
                nc.scalar.copy(q_sb, q_ps)

                # ---- k,v projection: 6 chunks of 4 tokens ----
                kvp = kvpp.tile([P, 6, 4, 2, 32], f32, tag="kvp")
                for j in range(6):
                    kT_ps = tps.tile([128, 128], f32, tag="tp")
                    nc.tensor.transpose(kT_ps, kv_t[:, j * 128:(j + 1) * 128],
                                        ident_sb)
                    kT_sb = trsb.tile([128, 128], f32, tag="kT")
                    nc.scalar.copy(kT_sb, kT_ps)
                    nc.tensor.matmul(kvp[:, j, :, :, :], lhsT=kT_sb, rhs=wkv_sb)

                # views into kvp: t-stride 64, k at +0, v at +32 (elements)
                k_ap = _ap(kvp, [[64, T], [8, H], [1, HD]])
                v_ap = _ap(kvp, [[64, T], [8, H], [1, HD]], extra_offset=32)

                # ---- scores = sum_d q*k ----
                prod = bigs.tile([P, T, H, HD], f32, tag="prod")
                qb_ap = _ap(q_sb, [[0, T], [8, H], [1, HD]])
                nc.vector.tensor_mul(prod, k_ap, qb_ap)
                scores = smalls.tile([P, T, H], f32, tag="scores")
                nc.vector.reduce_sum(scores, prod, axis=AX.X)

                # ---- softmax over t (no max-sub; |scores| <~ 5) ----
                exps = smalls.tile([P, T, H], f32, tag="exps")
                nc.scalar.activation(exps, scores, AF.Exp)
                denom = smalls.tile([P, H], f32, tag="denom")
                nc.vector.reduce_sum(denom, _ap(exps, [[1, H], [H, T]]), axis=AX.X)
                rden = smalls.tile([P, H], f32, tag="rden")
                nc.vector.reciprocal(rden, denom)

                # ---- ctx = sum_t attn*v (normalize at the end) ----
                prod2 = bigs.tile([P, T, H, HD], f32, tag="prod2")
                eb_ap = _ap(exps, [[H, T], [1, H], [0, HD]])
                nc.vector.tensor_mul(prod2, v_ap, eb_ap)
                ctxu = smalls.tile([P, A], f32, tag="ctxu")
                nc.vector.reduce_sum(ctxu, _ap(prod2, [[1, A], [A, T]]), axis=AX.X)
                ctx = smalls.tile([P, A], f32, tag="ctx")
                nc.vector.tensor_mul(ctx, ctxu, _ap(rden, [[1, H], [0, HD]]))

                # ---- out projection ----
                ctxT_ps = tps.tile([32, 128], f32, tag="tp")
                nc.tensor.transpose(ctxT_ps, ctx, ident_sb)
                ctxT_sb = trsb.tile([32, 128], f32, tag="ctxT")
                nc.scalar.copy(ctxT_sb, ctxT_ps)
                out1_ps = mms.tile([P, O], f32, tag="mm")
                nc.tensor.matmul(out1_ps, lhsT=ctxT_sb, rhs=wo_sb)

                # ---- LayerNorm ----
                st6 = smalls.tile([P, 6], f32, tag="st6")
                nc.vector.bn_stats(out=st6, in_=out1_ps)
                mv = smalls.tile([P, 2], f32, tag="mv")
                nc.vector.bn_aggr(out=mv, in_=st6)
                std = smalls.tile([P, 1], f32, tag="std")
                nc.scalar.activation(std, mv[:, 1:2], AF.Sqrt, bias=eps_sb)
                rstd = smalls.tile([P, 1], f32, tag="rstd")
                nc.vector.reciprocal(rstd, std)
                xc = smalls.tile([P, O], f32, tag="xc")
                nc.vector.tensor_scalar(xc, out1_ps, mv[:, 0:1], None,
                                        op0=OP.subtract)
                ln1 = smalls.tile([P, O], f32, tag="ln1")
                nc.vector.scalar_tensor_tensor(ln1, in0=xc, scalar=rstd,
                                               in1=lnw_sb, op0=OP.mult,
                                               op1=OP.mult)
                ln_out = smalls.tile([P, O], f32, tag="ln_out")
                nc.vector.tensor_add(ln_out, ln1, lnb_sb)

                # ---- MLP: gelu(ln @ Wd1) -> gelu(. @ Wd2), residual ----
                lnT_ps = tps.tile([32, 128], f32, tag="tp")
                nc.tensor.transpose(lnT_ps, ln_out, ident_sb)
                lnT_sb = trsb.tile([32, 128], f32, tag="lnT")
                nc.scalar.copy(lnT_sb, lnT_ps)
                h1_ps = mms.tile([P, O], f32, tag="mm")
                nc.tensor.matmul(h1_ps, lhsT=lnT_sb, rhs=wd1_sb)
                h1_sb = smalls.tile([P, O], f32, tag="h1")
                nc.scalar.activation(h1_sb, h1_ps, AF.Gelu)

                h1T_ps = tps.tile([32, 128], f32, tag="tp")
                nc.tensor.transpose(h1T_ps, h1_sb, ident_sb)
                h1T_sb = trsb.tile([32, 128], f32, tag="h1T")
                nc.scalar.copy(h1T_sb, h1T_ps)
                h2_ps = mms.tile([P, O], f32, tag="mm")
                nc.tensor.matmul(h2_ps, lhsT=h1T_sb, rhs=wd2_sb)
                h2_sb = smalls.tile([P, O], f32, tag="h2")
                nc.scalar.activation(h2_sb, h2_ps, AF.Gelu)

                outf = smalls.tile([P, O], f32, tag="outf")
                nc.vector.tensor_add(outf, ln_out, h2_sb)
                nc.sync.dma_start(out=out_d[i * P:(i + 1) * P, :], in_=outf)

    nc.compile()
    return nc


def _prep_weights(Wq, Wk, Wv, Wo, ln_w, ln_b, Wd1, Wd2):
    s = 1.0 / math.sqrt(HD)
    wq = np.zeros((16, 32), np.float32)
    wq[:DQ] = np.asarray(Wq, np.float32) * s
    wkv = np.zeros((128, 256), np.float32)
    Wk = np.asarray(Wk, np.float32)
    Wv = np.asarray(Wv, np.float32)
    for tl in range(4):
        wkv[tl * 32:(tl + 1) * 32, tl * 64:tl * 64 + 32] = Wk
        wkv[tl * 32:(tl + 1) * 32, tl * 64 + 32:tl * 64 + 64] = Wv
    lnw = np.broadcast_to(np.asarray(ln_w, np.float32), (128, 32)).copy()
    lnb = np.broadcast_to(np.asarray(ln_b, np.float32), (128, 32)).copy()
    return {
        "ident": np.eye(128, dtype=np.float32),
        "wq": wq,
        "wkv": wkv,
        "wo": np.ascontiguousarray(np.asarray(Wo, np.float32)),
        "wd1": np.ascontiguousarray(np.asarray(Wd1, np.float32)),
        "wd2": np.ascontiguousarray(np.asarray(Wd2, np.float32)),
        "lnw": lnw,
        "lnb": lnb,
    }


def kernel(query, kv, Wq, Wk, Wv, Wo, ln_w, ln_b, Wd1, Wd2):
    from concourse.bass_utils import run_bass_kernel_spmd

    if "nc" not in _CACHE:
        _CACHE["nc"] = _build()
    nc = _CACHE["nc"]

    query = np.ascontiguousarray(np.asarray(query, np.float32))
    kv = np.ascontiguousarray(np.asarray(kv, np.float32)).reshape(B, T * DKV)
    w = _prep_weights(Wq, Wk, Wv, Wo, ln_w, ln_b, Wd1, Wd2)

    in_maps = []
    for c in range(NCORES):
        m = dict(w)
        m["q_in"] = query[c * BP:(c + 1) * BP]
        m["kv_in"] = kv[c * BP:(c + 1) * BP]
        in_maps.append(m)

    res = run_bass_kernel_spmd(nc, in_maps, core_ids=list(range(NCORES)),
                               trace=False)
    _CACHE["last_results"] = res
    return np.concatenate([r["out"] for r in res.results], axis=0)
